# revision 29
# baseline (speedup 1.0000x reference)
"""Trainium2 Bass kernel for nn_Attention_481036337444.

Dense single-layer attention: 1x1-conv QKV projection, 4 heads x 32 dims over
4096 pixels (64x64), softmax attention, 1x1-conv output projection.

Sharding: 16 (batch, head) pairs over 8 cores -> core c handles batch c//2 and
heads {2*(c%2), 2*(c%2)+1}. Host divides by the softmax denominators and sums
the per-core partial projections (+bias). No collectives.

The kernel is PSUM-evacuation bound: only ScalarE (ACT, ~0.98 cols/ns
incl. overhead) and VectorE (DVE, ~0.87 cols/ns from PSUM) can read PSUM,
and evacuating exp(sim) for the 2x4096x4096 logit matrix dominates
(262144 columns/core). Design (295us baseline -> ~204us):
  - exp SPLIT across both engines: ACT does true exp via its LUT; DVE does
    a Schraudolph exp (single tensor_scalar add -> int16 -> bitcast fp16,
    ~1.7% rms multiplicative noise on half the j-blocks; measured final
    rel err 3.5e-3 vs the 2e-2 gate). The q projection is pre-scaled by
    1024*log2(e)*SCALE so PSUM sims are already in the Schraudolph domain;
    ACT recovers exp(s) via its free per-instruction scale.
  - fp16 datapath (x, q, k, v, exp weights, headout, po): halves logit
    quantization noise vs bf16.
  - per half-step (ic, p, hf): 16 j-blocks as 4 superbursts of 4
    row-band-concurrent K=32 sim MMs into two 2-bank PSUM tiles (ring of
    3); DVE evacuates the first tile, ACT the second. Two attn pairs are
    interleaved after each superburst so the strict-FIFO PE queue always
    has ready work while the ring round-trips (keeps HAM at 2.4GHz).
  - attn@v per (ic, head): even jb -> rows 0:33 (tile col 0), odd jb ->
    rows 64:97 (tile col 64), col-concurrent chains, ones-row gives the
    denominator halves for free. (Two row-banded matmuls accumulating
    into the SAME psum region abort the device -- hence the layout.)
  - acc evacuation is ONE 97-row ACT copy (junk rows 33:64 are finite and
    hit zero weights); the K=97 oproj matmul merges the even/odd halves
    and projects in one MM. oproj and po-evacuation are emitted 1-2 steps
    after the acc copy so their latency chains hide behind queued work.
  - softmax division on HOST from the exported fp16 denominator halves.
  - ~4us of zero matmuls at start warm the PE HAM clock gate and zero the
    acc-pool banks (needed for the 97-row copy trick).
Note: run-to-run HW time is bimodal (~204us at 2.4GHz vs ~244us when the
chip thermally enters P0 at ~2.0GHz); identical NEFF, alternates on
back-to-back runs.
"""

import numpy as np

F16 = np.float16
HEADS = 4
DIM_HEAD = 32
SCALE = DIM_HEAD ** -0.5
L2E = 1.4426950408889634
QSCALE = SCALE * 1024.0 * L2E       # PSUM sim = s * 1024*log2(e)
ACT_SCALE = 1.0 / (1024.0 * L2E)    # ACT: exp(scale * y) = exp(s)
SCHRAU_B = 15360.0 - 44.0           # i16 = y + B; bitcast fp16 ~ exp(s)
P = 128      # partitions == channels
N = 4096     # pixels = 64*64
CH = 512     # i-chunk width
NCH = N // CH
NCORES = 8

# exp structure per half-step: 16 j-blocks in 4 superbursts of 4 (each
# superburst = 4 row-band-concurrent sim MMs into TWO 2-bank PSUM ring
# tiles; ring of 3 tiles). Each superburst's two 2-jb halves are
# evacuated by ACT (true exp) and DVE (schraudolph) respectively; attn
# pairs of the previous half-step are interleaved between bursts so the
# strict-FIFO PE queue never stalls on the ring.
NETB = 3          # eT ring depth

_NC_CACHE = {}


def _build_nc():
    from concourse import bacc, mybir
    from concourse.tile import TileContext

    f32 = mybir.dt.float32
    f16 = mybir.dt.float16
    i16 = mybir.dt.int16
    EXP = mybir.ActivationFunctionType.Exp

    nc = bacc.Bacc()
    x_ext = nc.declare_dram_parameter("x", [P, N], f16, isOutput=False)
    # wmisc cols: [0:256] wq_rep(x QSCALE), [256:320] wk_t, [320:384] wv_t,
    # [384:512] wo_h0 (rows 0:33 and 64:97), [512:640] wo_h1
    wm_ext = nc.declare_dram_parameter("wmisc", [P, 640], f16, isOutput=False)
    # po cols: ic*1024 + 512*p : unnormalized per-head projected outputs
    po_ext = nc.declare_dram_parameter("po", [P, 2 * N], f16, isOutput=True)
    # denominator halves: rows (2*p + half) = headout row {32,96} of head p
    dd_ext = nc.declare_dram_parameter("dden", [4, N], f16, isOutput=True)

    with TileContext(nc) as tc:
        with (
            tc.tile_pool(name="persist", bufs=1) as persist,
            tc.tile_pool(name="sbB", bufs=2) as sbB,
            tc.tile_pool(name="ps", space="PSUM", bufs=3) as ps,
        ):
            # ---- warmup: ACT exp table load + PE clock off critical path
            wup = persist.tile([P, 8], f32)
            nc.vector.memset(wup[:], 0.0)
            wup2 = persist.tile([P, 8], f32)
            nc.scalar.activation(out=wup2[:], in_=wup[:], func=EXP)
            wupb = persist.tile([P, 8], f16)
            nc.vector.memset(wupb[:], 0.0)

            xt = persist.tile([P, N], f16)
            nc.sync.dma_start(out=xt[:, 0:CH], in_=x_ext[:, 0:CH])
            wmisc = persist.tile([P, 640], f16)
            nc.sync.dma_start(out=wmisc[:], in_=wm_ext[:])
            nc.sync.dma_start(out=xt[:, CH:N // 2], in_=x_ext[:, CH:N // 2])
            nc.sync.dma_start(out=xt[:, N // 2:], in_=x_ext[:, N // 2:])
            wq_rep = wmisc[:, 0:256]
            wk_t = wmisc[:, 256:320]
            wv_t = wmisc[:, 320:384]

            q4 = [persist.tile([P, N], f16, name=f"q4_{p}") for p in range(2)]
            k4 = [persist.tile([P, 1024], f16, name=f"k4_{p}")
                  for p in range(2)]
            # vTm: [128, 32 jb, 66]: per jb, cols 0:33 = head0 [v^T | 1],
            # cols 33:66 = head1
            vTm = persist.tile([P, 32 * 66], f16)
            vTr = vTm.rearrange("a (j m) -> a j m", m=66)
            nc.vector.memset(vTr[:, :, 32:33], 1.0)
            nc.vector.memset(vTr[:, :, 65:66], 1.0)
            vTp = vTm.rearrange("a (j p m) -> a j p m", p=2, m=33)
            # headout: cols 4096*p + ic*512; rows 0:33 even-half (+den@32),
            # rows 64:97 odd-half (+den@96). rows 33:64, 97:128 unused.
            headout = persist.tile([P, 2 * N], f16)
            eT = [persist.tile([P, 16 * CH], f16, name=f"expT{h}")
                  for h in range(NETB)]

            # x columns as [b(2), u(4), t(4), j(128)]: col = 2048b+512u+128t+j
            xr = xt.rearrange("c (b u t j) -> c b u t j", b=2, u=4, t=4, j=128)


            def emit_q4(p, ic):
                pq = ps.tile([P, 2 * CH], f32, tag="stage", name="pq")
                nc.tensor.matmul(
                    out=pq[:, 0:CH],
                    lhsT=wq_rep[:, p * 128:(p + 1) * 128],
                    rhs=xt[:, ic * CH:(ic + 1) * CH],
                    tile_position=(0, 0),
                )
                if p == 0:
                    nc.scalar.copy(q4[p][:, ic * CH:(ic + 1) * CH],
                                   pq[:, 0:CH])
                else:
                    nc.vector.tensor_copy(q4[p][:, ic * CH:(ic + 1) * CH],
                                          pq[:, 0:CH])

            def emit_k4(p, hfs=(0, 1)):
                for hf in hfs:
                    pk = ps.tile([P, 2 * CH], f32, tag="stage", name="pk")
                    for t in range(4):
                        nc.tensor.matmul(
                            out=pk[32 * t:32 * t + 32, 0:CH],
                            lhsT=wk_t[:, 32 * p:32 * p + 32],
                            rhs=xr[:, hf, :, t, :],
                            tile_position=(0, 32 * t),
                        )
                    nc.vector.tensor_copy(k4[p][:, hf * CH:(hf + 1) * CH],
                                          pk[:, 0:CH])

            def emit_vt(gs):
                for g in gs:
                    pv = ps.tile([P, 2 * CH], f32, tag="stage", name="pv")
                    for j in range(8):
                        jb = 8 * g + j
                        nc.tensor.matmul(
                            out=pv[:, 64 * j:64 * j + 64],
                            lhsT=xt[:, 128 * jb:128 * jb + 128],
                            rhs=wv_t[:],
                            tile_position=(0, 0),
                        )
                    pvr = pv[:, 0:CH].rearrange("a (j p m) -> a j p m",
                                                p=2, m=32)
                    nc.vector.tensor_copy(
                        vTp[:, 8 * g:8 * g + 8, :, 0:32], pvr)

            accs = {}
            pending_attn = None

            def emit_simburst(s, ic, p, hf, b):
                # 4 concurrent sim MMs (jb 4b..4b+3, distinct row bands)
                # split across two 2-bank ring tiles; ACT evacuates the
                # first (true exp), DVE the second (schraudolph). The ACT
                # op is emitted right after its two MMs so a ring stall on
                # the second tile does not delay it.
                buf = eT[s % NETB]
                sga = ps.tile([P, 2 * CH], f32, tag="stage", name="sga")
                sgb = ps.tile([P, 2 * CH], f32, tag="stage", name="sgb")
                off = 4 * b
                for k in range(2):
                    jb = 16 * hf + 4 * b + k
                    t, u = jb % 4, jb // 4
                    nc.tensor.matmul(
                        out=sga[:, k * CH:(k + 1) * CH],
                        lhsT=k4[p][32 * t:32 * t + 32,
                                   128 * u:128 * u + 128],
                        rhs=q4[p][32 * t:32 * t + 32,
                                  ic * CH:(ic + 1) * CH],
                        tile_position=(32 * t, 0),
                    )
                nc.vector.tensor_scalar_add(
                    buf[:, off * CH:(off + 2) * CH].bitcast(i16),
                    sga[:, 0:2 * CH], SCHRAU_B)
                for k in range(2, 4):
                    jb = 16 * hf + 4 * b + k
                    t, u = jb % 4, jb // 4
                    nc.tensor.matmul(
                        out=sgb[:, (k % 2) * CH:(k % 2 + 1) * CH],
                        lhsT=k4[p][32 * t:32 * t + 32,
                                   128 * u:128 * u + 128],
                        rhs=q4[p][32 * t:32 * t + 32,
                                  ic * CH:(ic + 1) * CH],
                        tile_position=(32 * t, 0),
                    )
                nc.scalar.activation(
                    out=buf[:, (off + 2) * CH:(off + 4) * CH],
                    in_=sgb[:, 0:2 * CH], func=EXP, scale=ACT_SCALE)

            def emit_attn_pairs(s, ic, p, hf, pairs):
                buf = eT[s % NETB]
                if hf == 0 and pairs[0] == 0:
                    accs[p] = ps.tile([P, CH], f32, tag="acc", bufs=2,
                                      name="acc")
                acc = accs[p]
                for pr in pairs:
                    for jbl in (2 * pr, 2 * pr + 1):
                        jb = 16 * hf + jbl
                        col = 0 if jb % 2 == 0 else 64
                        nc.tensor.matmul(
                            out=acc[col:col + 33, :],
                            lhsT=vTp[:, jb, p, :],
                            rhs=buf[:, jbl * CH:(jbl + 1) * CH],
                            tile_position=(0, col),
                            start=(jb < 2),
                            stop=(jb >= 30),
                            skip_group_check=True,
                        )

            pend_oproj = []
            pend_po = []

            def emit_acc_evac(ic, p):
                # after (ic, p)'s chains complete: evacuate raw acc (+den
                # rows) to headout. The dependent oproj matmul and po
                # evacuation are emitted 1 and 2 steps later so their
                # latency chains hide behind queued engine work.
                acc = accs[p]
                hcol = N * p + ic * CH
                # one 97-row copy; rows 33:64 carry finite junk that hits
                # zero weights in the K=97 oproj matmul
                nc.scalar.copy(headout[0:97, hcol:hcol + CH], acc[0:97])
                pend_oproj.append((ic, p))

            def emit_oproj():
                while pend_oproj:
                    ic, p = pend_oproj.pop(0)
                    hcol = N * p + ic * CH
                    po = ps.tile([P, CH], f32, tag="acc", bufs=2, name="po")
                    wo = wmisc[:, 384 + 128 * p:512 + 128 * p]
                    # single K=97 matmul merges even (rows 0:32) and odd
                    # (64:96) halves; rows 32:64 + 96 contribute zero
                    nc.tensor.matmul(
                        out=po[:, 0:CH], lhsT=wo[0:97, :],
                        rhs=headout[0:97, hcol:hcol + CH],
                        tile_position=(0, 0),
                    )
                    pend_po.append((ic, p, po))

            def emit_po_evac():
                while pend_po:
                    ic, p, po = pend_po.pop(0)
                    ob = sbB.tile([P, CH], f16, tag="outbuf", name="ob")
                    if p == 0:
                        nc.vector.tensor_copy(ob[:], po[:, 0:CH])
                    else:
                        nc.scalar.copy(ob[:], po[:, 0:CH])
                    nc.sync.dma_start(
                        out=po_ext[:, ic * 1024 + CH * p:
                                   ic * 1024 + CH * (p + 1)],
                        in_=ob[:],
                    )

            # ---- pre-pipeline: ~4us of dense dummy matmuls run under the
            # x DMA to take the PE HAM clock gate to 2.4GHz before the
            # pipeline starts
            wupw = persist.tile([P, CH], f16)
            nc.vector.memset(wupw[:], 0.0)
            def emit_warm():
                pwu = ps.tile([P, CH], f32, tag="acc", bufs=2, name="pwu")
                for i in range(5):
                    nc.tensor.matmul(out=pwu[0:128, 0:CH],
                                     lhsT=wupw[:, 0:128],
                                     rhs=wupw[:, 0:CH],
                                     tile_position=(0, 0),
                                     start=(i == 0), stop=(i == 4),
                                     skip_group_check=True)

            emit_warm()
            emit_q4(0, 0)
            emit_k4(0)
            emit_warm()

            steps = [(ic, p, hf) for ic in range(NCH) for p in range(2)
                     for hf in range(2)]
            for s, (ic, p, hf) in enumerate(steps):
                for b in range(4):
                    emit_simburst(s, ic, p, hf, b)
                    if s > 0:
                        aic, ap, ahf = pending_attn
                        emit_attn_pairs(s - 1, aic, ap, ahf, (2 * b,
                                                             2 * b + 1))
                    if s == len(steps) - 1 and b >= 2:
                        emit_attn_pairs(s, ic, p, hf, (2 * (b - 2),
                                                       2 * (b - 2) + 1))
                    if b == 0:
                        if s == 0:
                            emit_vt((0, 1))
                            emit_q4(1, 0)
                        if s == 1:
                            emit_vt((2,))
                            emit_k4(1, hfs=(0,))
                        if s == 2:
                            emit_vt((3,))
                            emit_k4(1, hfs=(1,))
                    if b == 1:
                        emit_oproj()
                    if b == 2 and hf == 0 and ic + 1 < NCH:
                        emit_q4(p, ic + 1)  # prefetch next chunk's q slice
                    if b == 3:
                        emit_po_evac()
                if s > 0 and pending_attn[2] == 1:
                    emit_acc_evac(pending_attn[0], pending_attn[1])
                pending_attn = (ic, p, hf)
            # pipeline flush
            s = len(steps) - 1
            aic, ap, ahf = pending_attn
            emit_attn_pairs(s, aic, ap, ahf, tuple(range(4, 8)))
            emit_acc_evac(aic, ap)
            emit_oproj()
            emit_po_evac()
            # denominator halves out
            for p in range(2):
                for half, row in ((0, 32), (1, 96)):
                    nc.sync.dma_start(
                        out=dd_ext[2 * p + half:2 * p + half + 1, :],
                        in_=headout[row:row + 1, N * p:N * (p + 1)],
                    )

    nc.finalize()
    return nc


def _get_nc():
    if "nc" not in _NC_CACHE:
        _NC_CACHE["nc"] = _build_nc()
    return _NC_CACHE["nc"]


def _prep_core(x, w_qkv, w_out, c):
    b, s = divmod(c, 2)
    h0 = 2 * s
    xc = np.ascontiguousarray(x[b].reshape(P, N)).astype(F16)
    wmisc = np.zeros((P, 640), np.float32)
    for p in range(2):
        h = h0 + p
        wq = w_qkv[32 * h:32 * h + 32, :]
        wk = w_qkv[128 + 32 * h:128 + 32 * h + 32, :]
        wv = w_qkv[256 + 32 * h:256 + 32 * h + 32, :]
        wmisc[:, 128 * p:128 * (p + 1)] = np.tile(
            (wq.T * QSCALE).astype(np.float32), (1, 4))
        wmisc[:, 256 + 32 * p:256 + 32 * (p + 1)] = wk.T
        wmisc[:, 320 + 32 * p:320 + 32 * (p + 1)] = wv.T
        # wo for head p, rows 0:32 and 64:96 (merges even/odd chain halves)
        wo = w_out[:, 32 * h:32 * h + 32].T     # [32, 128]
        wmisc[0:32, 384 + 128 * p:512 + 128 * p] = wo
        wmisc[64:96, 384 + 128 * p:512 + 128 * p] = wo
    return {"x": xc, "wmisc": wmisc.astype(F16)}


def _run(in_maps, trace=False):
    from concourse.bass_utils import run_bass_kernel_spmd
    nc = _get_nc()
    return run_bass_kernel_spmd(nc, in_maps, core_ids=list(range(NCORES)),
                                trace=trace)


def kernel(**inputs):
    x = np.asarray(inputs["x"], np.float32)
    w_qkv = np.asarray(inputs["w_qkv"], np.float32)
    w_out = np.asarray(inputs["w_out"], np.float32)
    b_out = np.asarray(inputs["b_out"], np.float32)

    in_maps = [_prep_core(x, w_qkv, w_out, c) for c in range(NCORES)]
    res = _run(in_maps)
    B = x.shape[0]
    out = np.empty((B, P, 64, 64), np.float32)
    for b in range(B):
        o = np.zeros((P, N), np.float64)
        for s in range(2):
            r = res.results[2 * b + s]
            po = np.asarray(r["po"], np.float32).reshape(P, NCH, 2, CH)
            dd = np.asarray(r["dden"], np.float32)
            for p in range(2):
                den = (dd[2 * p] + dd[2 * p + 1]).reshape(NCH, CH)
                o += (po[:, :, p, :] / den[None, :, :]).reshape(P, N)
        o = o + b_out[:, None]
        out[b] = o.reshape(P, 64, 64).astype(np.float32)
    return out
